# revision 38
# baseline (speedup 1.0000x reference)
"""Trainium2 Bass kernel for nn_Attention_65455301591248.

Multi-head attention: B=32, C=768, H=12 heads, S=512, D=64.
  q/k/v = W{q,k,v} @ x + b   (1x1 conv == channel GEMM), head-minor channel
  scores[k,h,q] = (q.k)/sqrt(D) + mask[k,q];  softmax over k
  attn = w @ v; concat head-major; out = Wo @ attn + bo

Sharding: pure data parallel over batch — 4 batches per core x 8 cores,
no collectives.

Per-core kernel strategy (all intermediates SBUF-resident):
  - Host pre-transposes weights (lhsT layout) and permutes Q/K/V output
    channels head-major (c' = h*64+d) so each head is a contiguous
    64-partition slab of the projection output. Wq/bq pre-scaled by 1/8.
  - scoresT[k,q] = k_h^T . q_h on PE (K=64; even/odd heads sit on row
    groups 0/64 -> concurrent row-tiled matmuls) into WIDE 2-bank PSUM
    tiles [128,1024] so each ACT exp eviction covers 2 k-chunks.
  - softmax over k (= partitions) without max subtraction (scores are
    in ~[-7, 7]; exp fits fp16 with headroom).  Numerator is
    exp(s)*exp(m): one wide exp(mask) [128,2048] per batch, one wide
    es*em multiply [128,2048] per head on DVE.
  - attn = V^T-block^T @ w via PE with a ones-column appended to each
    head's V^T slab: PSUM row 64 accumulates the softmax denominator
    for free.  One reciprocal_approx_fast per batch + GpSimd
    partition_broadcast + DVE mult normalizes into the head-major
    concat buffer (off the critical path - overlaps the next batch).
  - V bias is added by the PSUM eviction (tensor_add against a host-
    replicated [128,C] bias tile) instead of a K=1 matmul.
  - o_proj consumes the concat buffer; ACT adds biases on every
    PSUM->SBUF eviction.
  - Emission order interleaves projection / o_proj matmuls BETWEEN the
    scores/attn matmuls of the current head-pair so the in-order PE
    queue never stalls on the exp->mul chain cooking on ScalarE/VectorE.
"""

import numpy as np

try:
    import concourse.bass as bass  # noqa: F401
except ImportError:  # pragma: no cover
    import sys

    sys.path.insert(0, "/opt/trn_rl_repo")

import concourse.bass as bass
import concourse.tile as tile
from concourse import bacc, mybir
from concourse.bass_utils import run_bass_kernel_spmd

B, C, H, S, D = 32, 768, 12, 512, 64
NCORES = 8
NB = B // NCORES  # batches per core
F16 = mybir.dt.float16
F32 = mybir.dt.float32
NC_CHUNKS = C // 128  # 6
NK_CHUNKS = S // 128  # 4
VROW = H * (D + 1)  # 780: per-head 64 v columns + 1 ones column

_COMPILED = None


def _build():
    """Build + compile the per-core Bass program (runs on each of 8 cores)."""
    nc = bacc.Bacc("TRN2", target_bir_lowering=False, debug=False)

    x_d = nc.dram_tensor("x", [NB, C, S], F16, kind="ExternalInput")
    m_d = nc.dram_tensor("mask", [NB, S, S], F16, kind="ExternalInput")
    wq_d = nc.dram_tensor("wqt", [C, C], F16, kind="ExternalInput")
    wk_d = nc.dram_tensor("wkt", [C, C], F16, kind="ExternalInput")
    wv_d = nc.dram_tensor("wvt", [C, C], F16, kind="ExternalInput")
    wo_d = nc.dram_tensor("wot", [C, C], F16, kind="ExternalInput")
    # packed per-partition biases: cols 0-5 bq/8, 6-11 bk, 12-17 bo
    bcol_d = nc.dram_tensor("bcols", [128, 18], F32, kind="ExternalInput")
    # v bias replicated across partitions (head-major channel order)
    bvbc_d = nc.dram_tensor("bvbc", [128, C], F32, kind="ExternalInput")
    y_d = nc.dram_tensor("y", [NB, C, S], F16, kind="ExternalOutput")

    with tile.TileContext(nc) as tc:
        with (
            tc.tile_pool(name="wpool", bufs=1) as wpool,
            tc.tile_pool(name="const", bufs=1) as const,
            tc.tile_pool(name="xp", bufs=2) as xp,
            tc.tile_pool(name="qk", bufs=2) as qk,
            tc.tile_pool(name="vp", bufs=2) as vp,
            tc.tile_pool(name="mp", bufs=2) as mp,
            tc.tile_pool(name="wexp", bufs=2) as wexp,
            tc.tile_pool(name="cat", bufs=2) as cat,
            tc.tile_pool(name="op", bufs=2) as op,
            tc.tile_pool(name="rp", bufs=2) as rp,
            tc.tile_pool(name="ps_proj", bufs=3, space="PSUM") as ps_proj,
            tc.tile_pool(name="ps_s", bufs=2, space="PSUM") as ps_s,
            tc.tile_pool(name="ps_a", bufs=1, space="PSUM") as ps_a,
        ):
            # ---- persistent weights / constants -------------------------
            def load_w(w_d, name):
                tiles = []
                for j in range(NC_CHUNKS):
                    t = wpool.tile([128, C], F16, tag=f"{name}{j}")
                    nc.sync.dma_start(out=t[:], in_=w_d.ap()[j * 128 : (j + 1) * 128, :])
                    tiles.append(t)
                return tiles

            # wq + batch-0 x first (interleaved) so the first projection
            # group can start as soon as its first operand pair lands
            # weight chunks alternate between the two HWDGE queues (sync /
            # Activation) so both streams transfer in parallel; x0 rides the
            # sync queue between them
            wq_sb, xt0 = [], []
            for j in range(NC_CHUNKS):
                t = wpool.tile([128, C], F16, tag=f"wq{j}", name=f"wq{j}")
                eng = nc.sync if j % 2 == 0 else nc.scalar
                eng.dma_start(out=t[:], in_=wq_d.ap()[j * 128 : (j + 1) * 128, :])
                wq_sb.append(t)
                tx = xp.tile([128, S], F16, tag=f"x{j}", name=f"x0_{j}")
                nc.sync.dma_start(
                    out=tx[:], in_=x_d.ap()[0, j * 128 : (j + 1) * 128, :]
                )
                xt0.append(tx)
            wk_sb = []
            for j in range(NC_CHUNKS):
                t = wpool.tile([128, C], F16, tag=f"wk{j}", name=f"wk{j}")
                eng = nc.scalar if j % 2 == 0 else nc.sync
                eng.dma_start(out=t[:], in_=wk_d.ap()[j * 128 : (j + 1) * 128, :])
                wk_sb.append(t)
            wv_sb = []
            for j in range(NC_CHUNKS):
                t = wpool.tile([128, C], F16, tag=f"wv{j}", name=f"wv{j}")
                eng = nc.sync if j % 2 == 0 else nc.scalar
                eng.dma_start(out=t[:], in_=wv_d.ap()[j * 128 : (j + 1) * 128, :])
                wv_sb.append(t)
            wo_sb = []  # loaded at the end of batch 0 (first needed in batch 1)
            # constants go on the Activation HWDGE queue so they don't delay
            # the weight/x stream on the sync queue
            bcol = const.tile([128, 18], F32, tag="bcol")
            nc.scalar.dma_start(out=bcol[:], in_=bcol_d.ap()[:, :])
            bvbc = const.tile([128, C], F32, tag="bvbc")
            nc.scalar.dma_start(out=bvbc[:], in_=bvbc_d.ap()[:, :])

            # ---------------------------------------------------------
            # Emission helpers.  Projection / o-proj work is packaged into
            # thunks (one PSUM group each) so it can be interleaved between
            # the attention matmuls of the PREVIOUS batch: the PE queue
            # then always holds independent matmuls while the exp/mult
            # chains of the current pair cook on ScalarE/VectorE.
            # ---------------------------------------------------------

            def load_x(b):
                xt = []
                for j in range(NC_CHUNKS):
                    t = xp.tile([128, S], F16, tag=f"x{j}", name=f"x{j}")
                    nc.sync.dma_start(
                        out=t[:], in_=x_d.ap()[b, j * 128 : (j + 1) * 128, :]
                    )
                    xt.append(t)
                return xt

            def em_thunk(b, em_out):
                # em = exp(mask), one wide [128, 4*S] tile: block kc holds
                # exp(mask[kc-chunk keys, all q]).  4 DMAs (on the Activation
                # HWDGE queue, off the weight/x stream) + ONE exp.
                def one():
                    mraw = mp.tile([128, 4 * S], F16, tag="mraw", name="mraw")
                    for kc in range(NK_CHUNKS):
                        nc.scalar.dma_start(
                            out=mraw[:, kc * S : (kc + 1) * S],
                            in_=m_d.ap()[b, kc * 128 : (kc + 1) * 128, :],
                        )
                    e = mp.tile([128, 4 * S], F16, tag="em", name="em")
                    nc.scalar.activation(
                        out=e[:], in_=mraw[:], func=mybir.ActivationFunctionType.Exp
                    )
                    em_out[0] = e

                return one

            def qk_group(w_sb, xt, bias_col, name, co, outs):
                ps = ps_proj.tile([128, S], F32, tag="proj", name="ps_p")
                for ki in range(NC_CHUNKS):
                    nc.tensor.matmul(
                        ps[:],
                        w_sb[ki][:, co * 128 : (co + 1) * 128],
                        xt[ki][:],
                        start=(ki == 0),
                        stop=(ki == NC_CHUNKS - 1),
                    )
                dt = qk.tile([128, S], F16, tag=f"{name}{co}", name=f"{name}{co}")
                if co % 2 == 0:
                    nc.vector.tensor_scalar_add(
                        dt[:], ps[:], bcol[:, bias_col + co : bias_col + co + 1]
                    )
                else:
                    nc.scalar.activation(
                        out=dt[:],
                        in_=ps[:],
                        func=mybir.ActivationFunctionType.Identity,
                        bias=bcol[:, bias_col + co : bias_col + co + 1],
                    )
                outs[co] = dt

            def v_group(xt, sc, half, v_out):
                # v^T projection chunk: out [s, c'] with per-head ones col.
                # Bias is added by the eviction (bvbc is host-replicated
                # across partitions) - no K=1 matmul needed.
                if half == 0:
                    vt = vp.tile([128, VROW], F16, tag=f"v{sc}", name=f"v{sc}")
                    vv = vt.rearrange("p (h w) -> p h w", w=D + 1)
                    nc.vector.memset(vv[:, :, D : D + 1], 1.0)
                    v_out[sc] = vt
                else:
                    vt = v_out[sc]
                    vv = vt.rearrange("p (h w) -> p h w", w=D + 1)
                hw = C // 2  # 384 = 6 heads
                ps = ps_proj.tile([128, hw], F32, tag="proj", name="ps_v")
                for ki in range(NC_CHUNKS):
                    nc.tensor.matmul(
                        ps[:],
                        xt[ki][:, sc * 128 : (sc + 1) * 128],
                        wv_sb[ki][:, half * hw : (half + 1) * hw],
                        start=(ki == 0),
                        stop=(ki == NC_CHUNKS - 1),
                    )
                nc.vector.tensor_add(
                    vv[:, half * 6 : (half + 1) * 6, 0:D],
                    ps.rearrange("p (h w) -> p h w", w=D),
                    bvbc.rearrange("p (h w) -> p h w", w=D)[
                        :, half * 6 : (half + 1) * 6, :
                    ],
                )

            def qkv_thunks(xt, q_out, k_out, v_out):
                th = []
                for co in range(NC_CHUNKS):
                    th.append(lambda co=co: qk_group(wq_sb, xt, 0, "q", co, q_out))
                for co in range(NC_CHUNKS):
                    th.append(lambda co=co: qk_group(wk_sb, xt, 6, "k", co, k_out))
                for sc in range(NK_CHUNKS):
                    for half in range(2):
                        th.append(
                            lambda sc=sc, half=half: v_group(xt, sc, half, v_out)
                        )
                return th

            def oproj_thunks(b, cat_sb):
                def one(co):
                    ps = ps_proj.tile([128, S], F32, tag="proj", name="ps_o")
                    for ki in range(NC_CHUNKS):
                        nc.tensor.matmul(
                            ps[:],
                            wo_sb[ki][:, co * 128 : (co + 1) * 128],
                            cat_sb[ki][:],
                            start=(ki == 0),
                            stop=(ki == NC_CHUNKS - 1),
                        )
                    ot = op.tile([128, S], F16, tag=f"o{co}", name="ot")
                    nc.scalar.activation(
                        out=ot[:],
                        in_=ps[:],
                        func=mybir.ActivationFunctionType.Identity,
                        bias=bcol[:, 12 + co : 13 + co],
                    )
                    nc.sync.dma_start(
                        out=y_d.ap()[b, co * 128 : (co + 1) * 128, :], in_=ot[:]
                    )

                return [lambda co=co: one(co) for co in range(NC_CHUNKS)]

            # ---- attention for one batch, weaving `work` thunks between
            # the scores/attn matmul groups -------------------------------
            def attention(b, q_sb, k_sb, v_sb, em_box, work, norm_lag=2):
                cat_sb = []
                for j in range(NC_CHUNKS):
                    ct = cat.tile([128, S], F16, tag=f"c{j}", name=f"cat{j}")
                    cat_sb.append(ct)
                dens = {}
                stgs = {}

                nwork = len(work)
                wi = 0

                def drain(target):
                    nonlocal wi
                    t = min(nwork, target)
                    while wi < t:
                        work[wi]()
                        wi += 1

                def scores_half(hp, khalf, es_pair):
                    # two kc chunks for both heads of the pair; j0 and j1
                    # land in different PSUM buffers (different banks) and
                    # different PE row groups -> the MM pairs overlap.
                    pj0 = ps_s.tile([128, 1024], F32, tag="s", name="ps_s0")
                    pj1 = ps_s.tile([128, 1024], F32, tag="s", name="ps_s1")
                    for i in range(2):
                        kc = 2 * khalf + i
                        for j, ps in ((0, pj0), (1, pj1)):
                            po = j * D
                            nc.tensor.matmul(
                                ps[:, i * 512 : (i + 1) * 512],
                                k_sb[hp][po : po + D, kc * 128 : (kc + 1) * 128],
                                q_sb[hp][po : po + D, :],
                                start=True,
                                stop=True,
                            )
                    for j, ps in ((0, pj0), (1, pj1)):
                        nc.scalar.activation(
                            out=es_pair[j][:, khalf * 1024 : (khalf + 1) * 1024],
                            in_=ps[:],
                            func=mybir.ActivationFunctionType.Exp,
                        )

                def attn_head(h, es):
                    psa = ps_a.tile([D + 1, S], F32, tag="attn", name="psa")
                    for kc in range(NK_CHUNKS):
                        nc.tensor.matmul(
                            psa[:],
                            v_sb[kc][:, h * (D + 1) : (h + 1) * (D + 1)],
                            es[:, kc * 512 : (kc + 1) * 512],
                            start=(kc == 0),
                            stop=(kc == NK_CHUNKS - 1),
                        )
                    # one-op eviction: rows 0-63 unnormalized attn + row 64
                    # denominator; a tiny casting DMA (gpsimd SWDGE is the
                    # only queue whose DMAs may cast) drops the fp16
                    # denominator into a partition-0 fp32 tile
                    stg = wexp.tile([D + 1, S], F16, tag=f"stg{h}", name="stg")
                    # evictions alternate DVE/ACT by head parity; each
                    # attn_head is emitted BEFORE the scores_half of its
                    # iteration, so the stg lands ahead of the wide muls
                    # (DVE) / the khalf exps (ACT) and the single psa bank
                    # turns over fast on both paths
                    if h % 2 == 0:
                        nc.vector.tensor_copy(stg[:], psa[:])
                    else:
                        nc.scalar.activation(
                            out=stg[:],
                            in_=psa[:],
                            func=mybir.ActivationFunctionType.Copy,
                        )
                    # fp16 per-pair denominator tile at base partition 0 (so
                    # the custom-DVE reciprocal is partition-aligned)
                    p, j = h // 2, h % 2
                    if j == 0:
                        dens[p] = rp.tile([2, S], F16, tag=f"den{p % 2}", name="den")
                    nc.sync.dma_start(
                        out=dens[p][j : j + 1, :], in_=stg[D : D + 1, :]
                    )
                    stgs[h] = stg

                def norm_pair(p):
                    # normalize the two heads of pair p into cat chunk p;
                    # woven into the SAME batch so the chain never head-of-
                    # line-blocks the next batch's DVE queue.  One per-pair
                    # reciprocal; rstage DMAs hop each row to partition 0
                    # for the gpsimd broadcast.  Both broadcasts issue
                    # before the first mul so the DVE only eats one
                    # broadcast latency.
                    denf = rp.tile([2, S], F32, tag=f"denf{p % 2}", name="denf")
                    nc.vector.tensor_copy(denf[:], dens[p][:])
                    rip = rp.tile([2, S], F32, tag=f"ri{p % 2}", name="rip")
                    nc.vector.reciprocal_approx_fast(out=rip[:], in_=denf[:])
                    riph = rp.tile([2, S], F16, tag=f"rih{p % 2}", name="riph")
                    nc.vector.tensor_copy(riph[:], rip[:])
                    rbs = []
                    for j in range(2):
                        rstage = rp.tile([1, S], F16, tag="rstage", name="rstage")
                        nc.sync.dma_start(out=rstage[:], in_=riph[j : j + 1, :])
                        rb = rp.tile([64, S], F16, tag="rb", name="rb")
                        nc.gpsimd.partition_broadcast(rb[:], rstage[:])
                        rbs.append(rb)
                    for j in range(2):
                        h = 2 * p + j
                        nc.vector.tensor_mul(
                            cat_sb[p][j * D : j * D + D, :],
                            stgs[h][0:D, :],
                            rbs[j][:],
                        )

                pend = None  # (hp, es0, es1) with muls already emitted
                for hp in range(H // 2):
                    es0 = wexp.tile([128, 4 * S], F16, tag="es0", name="es0")
                    es1 = wexp.tile([128, 4 * S], F16, tag="es1", name="es1")
                    if pend is not None:
                        attn_head(2 * pend[0], pend[1])
                    scores_half(hp, 0, (es0, es1))
                    drain((6 * hp + 2) * nwork // 36)
                    if pend is not None:
                        attn_head(2 * pend[0] + 1, pend[2])
                    scores_half(hp, 1, (es0, es1))
                    nc.vector.tensor_mul(es0[:], es0[:], em_box[0][:])
                    nc.vector.tensor_mul(es1[:], es1[:], em_box[0][:])
                    if hp >= norm_lag:
                        norm_pair(hp - norm_lag)
                    drain((6 * hp + 6) * nwork // 36)
                    pend = (hp, es0, es1)
                attn_head(2 * pend[0], pend[1])
                attn_head(2 * pend[0] + 1, pend[2])
                for p in range(H // 2 - norm_lag, H // 2):
                    norm_pair(p)
                drain(nwork)
                return cat_sb

            # ---- prologue: only what batch 0's first pairs need runs
            # un-interleaved (em, q0/k0/q1/k1, all of V); the rest of batch
            # 0's projections weave into attention(0) itself ---------------
            em_cur = [None]
            q_cur, k_cur, v_cur = {}, {}, {}
            em_thunk(0, em_cur)()
            qkv0 = qkv_thunks(xt0, q_cur, k_cur, v_cur)
            for i in (0, 6, 1, 7):  # q0, k0, q1, k1
                qkv0[i]()
            for th in qkv0[12:]:  # V groups
                th()
            wo_sb.extend(load_w(wo_d, "wo"))

            prev_cat = None
            # late-chunk q/k thunks carried into the NEXT batch's weave so
            # the filler-poor last batch stays fed
            deferred = [qkv0[2], qkv0[8], qkv0[3], qkv0[9], qkv0[4], qkv0[10], qkv0[5], qkv0[11]]
            for b in range(NB):
                work = list(deferred)
                deferred = []
                em_next = [None]
                q_next, k_next, v_next = {}, {}, {}
                if b + 1 < NB:
                    xt_next = load_x(b + 1)
                    qkv = qkv_thunks(xt_next, q_next, k_next, v_next)
                    # q0..2/k0..2 (needed earliest by batch b+1's scores),
                    # em in the middle, V groups after; q3..5/k3..5 are
                    # deferred into batch b+1's own weave (they drain there
                    # before pair 3 needs them)
                    for i in range(3):
                        work += [qkv[i], qkv[6 + i]]
                    work.append(em_thunk(b + 1, em_next))
                    work += qkv[12:]
                    deferred = [
                        qkv[3], qkv[9], qkv[4], qkv[10], qkv[5], qkv[11]
                    ]
                if prev_cat is not None:
                    work += oproj_thunks(b - 1, prev_cat)
                prev_cat = attention(
                    b, q_cur, k_cur, v_cur, em_cur, work,
                    norm_lag=2 if b + 1 < NB else 1,
                )
                em_cur, q_cur, k_cur, v_cur = em_next, q_next, k_next, v_next

            for th in oproj_thunks(NB - 1, prev_cat):
                th()

    nc.compile()
    return nc


def _get_compiled():
    global _COMPILED
    if _COMPILED is None:
        _COMPILED = _build()
    return _COMPILED


def _headmajor(wT):
    """Permute the output-channel axis of a transposed weight from the
    reference's head-minor order (c = d*H + h) to head-major (c' = h*D + d)."""
    return np.ascontiguousarray(
        wT.reshape(C, D, H).transpose(0, 2, 1).reshape(C, C)
    )


def _headmajor_b(bv):
    return np.ascontiguousarray(bv.reshape(D, H).T.reshape(C))


def prepare_in_maps(hidden_state, mask, Wq, bq, Wk, bk, Wv, bv, Wo, bo):
    x = np.asarray(hidden_state).reshape(B, C, S)
    m = np.asarray(mask).reshape(B, S, S)
    scale = np.float32(D**-0.5)

    wqt = np.ascontiguousarray(
        (_headmajor(np.asarray(Wq).T).astype(np.float32) * scale).astype(np.float16)
    )
    wkt = _headmajor(np.asarray(Wk).T)
    wvt = _headmajor(np.asarray(Wv).T)
    wot = np.ascontiguousarray(np.asarray(Wo).T)

    bq_s = (_headmajor_b(np.asarray(bq)).astype(np.float32) * scale).astype(np.float16)
    bk_p = _headmajor_b(np.asarray(bk))
    bo_p = np.asarray(bo)
    bcols = np.stack(
        [bq_s[j * 128 : (j + 1) * 128] for j in range(NC_CHUNKS)]
        + [bk_p[j * 128 : (j + 1) * 128] for j in range(NC_CHUNKS)]
        + [bo_p[j * 128 : (j + 1) * 128] for j in range(NC_CHUNKS)],
        axis=1,
    ).astype(np.float32)
    bvbc = np.ascontiguousarray(
        np.broadcast_to(
            _headmajor_b(np.asarray(bv)).astype(np.float32)[None, :], (128, C)
        )
    )

    shared = {
        "wqt": wqt,
        "wkt": wkt,
        "wvt": wvt,
        "wot": wot,
        "bcols": np.ascontiguousarray(bcols),
        "bvbc": bvbc,
    }
    in_maps = []
    for i in range(NCORES):
        sl = slice(i * NB, (i + 1) * NB)
        in_maps.append(
            dict(
                shared,
                x=np.ascontiguousarray(x[sl]),
                mask=np.ascontiguousarray(m[sl]),
            )
        )
    return in_maps


def kernel(**inputs):
    nc = _get_compiled()
    in_maps = prepare_in_maps(**inputs)
    res = run_bass_kernel_spmd(nc, in_maps, core_ids=list(range(NCORES)))
    y = np.concatenate([res.results[i]["y"] for i in range(NCORES)], axis=0)
    return y.reshape(B, C, 1, S)


# revision 39
# speedup vs baseline: 1.0283x; 1.0283x over previous
"""Trainium2 Bass kernel for nn_Attention_65455301591248.

Multi-head attention: B=32, C=768, H=12 heads, S=512, D=64.
  q/k/v = W{q,k,v} @ x + b   (1x1 conv == channel GEMM), head-minor channel
  scores[k,h,q] = (q.k)/sqrt(D) + mask[k,q];  softmax over k
  attn = w @ v; concat head-major; out = Wo @ attn + bo

Sharding: pure data parallel over batch — 4 batches per core x 8 cores,
no collectives.

Per-core kernel strategy (all intermediates SBUF-resident):
  - Host pre-transposes weights (lhsT layout) and permutes Q/K/V output
    channels head-major (c' = h*64+d) so each head is a contiguous
    64-partition slab of the projection output. Wq/bq pre-scaled by 1/8.
  - scoresT[k,q] = k_h^T . q_h on PE (K=64; even/odd heads sit on row
    groups 0/64 -> concurrent row-tiled matmuls) into WIDE 2-bank PSUM
    tiles [128,1024] so each ACT exp eviction covers 2 k-chunks.
  - softmax over k (= partitions) without max subtraction (scores are
    in ~[-7, 7]; exp fits fp16 with headroom).  Numerator is
    exp(s)*exp(m): one wide exp(mask) [128,2048] per batch, one wide
    es*em multiply [128,2048] per head on DVE.
  - attn = V^T-block^T @ w via PE with a ones-column appended to each
    head's V^T slab: PSUM row 64 accumulates the softmax denominator
    for free.  One reciprocal_approx_fast per batch + GpSimd
    partition_broadcast + DVE mult normalizes into the head-major
    concat buffer (off the critical path - overlaps the next batch).
  - V bias is added by the PSUM eviction (tensor_add against a host-
    replicated [128,C] bias tile) instead of a K=1 matmul.
  - o_proj consumes the concat buffer; ACT adds biases on every
    PSUM->SBUF eviction.
  - Emission order interleaves projection / o_proj matmuls BETWEEN the
    scores/attn matmuls of the current head-pair so the in-order PE
    queue never stalls on the exp->mul chain cooking on ScalarE/VectorE.
"""

import numpy as np

try:
    import concourse.bass as bass  # noqa: F401
except ImportError:  # pragma: no cover
    import sys

    sys.path.insert(0, "/opt/trn_rl_repo")

import concourse.bass as bass
import concourse.tile as tile
from concourse import bacc, mybir
from concourse.bass_utils import run_bass_kernel_spmd

B, C, H, S, D = 32, 768, 12, 512, 64
NCORES = 8
NB = B // NCORES  # batches per core
F16 = mybir.dt.float16
F32 = mybir.dt.float32
NC_CHUNKS = C // 128  # 6
NK_CHUNKS = S // 128  # 4
VROW = H * (D + 1)  # 780: per-head 64 v columns + 1 ones column

_COMPILED = None


def _build():
    """Build + compile the per-core Bass program (runs on each of 8 cores)."""
    nc = bacc.Bacc("TRN2", target_bir_lowering=False, debug=False)

    x_d = nc.dram_tensor("x", [NB, C, S], F16, kind="ExternalInput")
    m_d = nc.dram_tensor("mask", [NB, S, S], F16, kind="ExternalInput")
    wq_d = nc.dram_tensor("wqt", [C, C], F16, kind="ExternalInput")
    wk_d = nc.dram_tensor("wkt", [C, C], F16, kind="ExternalInput")
    wv_d = nc.dram_tensor("wvt", [C, C], F16, kind="ExternalInput")
    wo_d = nc.dram_tensor("wot", [C, C], F16, kind="ExternalInput")
    # packed per-partition biases: cols 0-5 bq/8, 6-11 bk, 12-17 bo
    bcol_d = nc.dram_tensor("bcols", [128, 18], F32, kind="ExternalInput")
    # v bias replicated across partitions (head-major channel order)
    bvbc_d = nc.dram_tensor("bvbc", [128, C], F32, kind="ExternalInput")
    y_d = nc.dram_tensor("y", [NB, C, S], F16, kind="ExternalOutput")

    with tile.TileContext(nc) as tc:
        with (
            tc.tile_pool(name="wpool", bufs=1) as wpool,
            tc.tile_pool(name="const", bufs=1) as const,
            tc.tile_pool(name="xp", bufs=2) as xp,
            tc.tile_pool(name="qk", bufs=2) as qk,
            tc.tile_pool(name="vp", bufs=2) as vp,
            tc.tile_pool(name="mp", bufs=2) as mp,
            tc.tile_pool(name="wexp", bufs=2) as wexp,
            tc.tile_pool(name="cat", bufs=2) as cat,
            tc.tile_pool(name="op", bufs=2) as op,
            tc.tile_pool(name="rp", bufs=2) as rp,
            tc.tile_pool(name="ps_proj", bufs=3, space="PSUM") as ps_proj,
            tc.tile_pool(name="ps_s", bufs=2, space="PSUM") as ps_s,
            tc.tile_pool(name="ps_a", bufs=1, space="PSUM") as ps_a,
        ):
            # ---- persistent weights / constants -------------------------
            def load_w(w_d, name):
                tiles = []
                for j in range(NC_CHUNKS):
                    t = wpool.tile([128, C], F16, tag=f"{name}{j}")
                    nc.sync.dma_start(out=t[:], in_=w_d.ap()[j * 128 : (j + 1) * 128, :])
                    tiles.append(t)
                return tiles

            # wq + batch-0 x first (interleaved) so the first projection
            # group can start as soon as its first operand pair lands
            # weight chunks alternate between the two HWDGE queues (sync /
            # Activation) so both streams transfer in parallel; x0 rides the
            # sync queue between them
            wq_sb, xt0 = [], []
            for j in range(NC_CHUNKS):
                t = wpool.tile([128, C], F16, tag=f"wq{j}", name=f"wq{j}")
                eng = nc.sync if j % 2 == 0 else nc.scalar
                eng.dma_start(out=t[:], in_=wq_d.ap()[j * 128 : (j + 1) * 128, :])
                wq_sb.append(t)
                tx = xp.tile([128, S], F16, tag=f"x{j}", name=f"x0_{j}")
                nc.sync.dma_start(
                    out=tx[:], in_=x_d.ap()[0, j * 128 : (j + 1) * 128, :]
                )
                xt0.append(tx)
            wk_sb = []
            for j in range(NC_CHUNKS):
                t = wpool.tile([128, C], F16, tag=f"wk{j}", name=f"wk{j}")
                eng = nc.scalar if j % 2 == 0 else nc.sync
                eng.dma_start(out=t[:], in_=wk_d.ap()[j * 128 : (j + 1) * 128, :])
                wk_sb.append(t)
            wv_sb = []
            for j in range(NC_CHUNKS):
                t = wpool.tile([128, C], F16, tag=f"wv{j}", name=f"wv{j}")
                eng = nc.sync if j % 2 == 0 else nc.scalar
                eng.dma_start(out=t[:], in_=wv_d.ap()[j * 128 : (j + 1) * 128, :])
                wv_sb.append(t)
            wo_sb = []  # loaded at the end of batch 0 (first needed in batch 1)
            # constants go on the Activation HWDGE queue so they don't delay
            # the weight/x stream on the sync queue
            bcol = const.tile([128, 18], F32, tag="bcol")
            nc.scalar.dma_start(out=bcol[:], in_=bcol_d.ap()[:, :])
            bvbc = const.tile([128, C], F32, tag="bvbc")
            nc.scalar.dma_start(out=bvbc[:], in_=bvbc_d.ap()[:, :])

            # ---------------------------------------------------------
            # Emission helpers.  Projection / o-proj work is packaged into
            # thunks (one PSUM group each) so it can be interleaved between
            # the attention matmuls of the PREVIOUS batch: the PE queue
            # then always holds independent matmuls while the exp/mult
            # chains of the current pair cook on ScalarE/VectorE.
            # ---------------------------------------------------------

            def load_x(b):
                xt = []
                for j in range(NC_CHUNKS):
                    t = xp.tile([128, S], F16, tag=f"x{j}", name=f"x{j}")
                    nc.sync.dma_start(
                        out=t[:], in_=x_d.ap()[b, j * 128 : (j + 1) * 128, :]
                    )
                    xt.append(t)
                return xt

            def em_thunk(b, em_out):
                # em = exp(mask), one wide [128, 4*S] tile: block kc holds
                # exp(mask[kc-chunk keys, all q]).  4 DMAs (on the Activation
                # HWDGE queue, off the weight/x stream) + ONE exp.
                def one():
                    mraw = mp.tile([128, 4 * S], F16, tag="mraw", name="mraw")
                    for kc in range(NK_CHUNKS):
                        nc.scalar.dma_start(
                            out=mraw[:, kc * S : (kc + 1) * S],
                            in_=m_d.ap()[b, kc * 128 : (kc + 1) * 128, :],
                        )
                    e = mp.tile([128, 4 * S], F16, tag="em", name="em")
                    nc.scalar.activation(
                        out=e[:], in_=mraw[:], func=mybir.ActivationFunctionType.Exp
                    )
                    em_out[0] = e

                return one

            def qk_group(w_sb, xt, bias_col, name, co, outs):
                ps = ps_proj.tile([128, S], F32, tag="proj", name="ps_p")
                for ki in range(NC_CHUNKS):
                    nc.tensor.matmul(
                        ps[:],
                        w_sb[ki][:, co * 128 : (co + 1) * 128],
                        xt[ki][:],
                        start=(ki == 0),
                        stop=(ki == NC_CHUNKS - 1),
                    )
                dt = qk.tile([128, S], F16, tag=f"{name}{co}", name=f"{name}{co}")
                if co % 2 == 0:
                    nc.vector.tensor_scalar_add(
                        dt[:], ps[:], bcol[:, bias_col + co : bias_col + co + 1]
                    )
                else:
                    nc.scalar.activation(
                        out=dt[:],
                        in_=ps[:],
                        func=mybir.ActivationFunctionType.Identity,
                        bias=bcol[:, bias_col + co : bias_col + co + 1],
                    )
                outs[co] = dt

            def v_group(xt, sc, half, v_out):
                # v^T projection chunk: out [s, c'] with per-head ones col.
                # Bias is added by the eviction (bvbc is host-replicated
                # across partitions) - no K=1 matmul needed.
                if half == 0:
                    vt = vp.tile([128, VROW], F16, tag=f"v{sc}", name=f"v{sc}")
                    vv = vt.rearrange("p (h w) -> p h w", w=D + 1)
                    nc.vector.memset(vv[:, :, D : D + 1], 1.0)
                    v_out[sc] = vt
                else:
                    vt = v_out[sc]
                    vv = vt.rearrange("p (h w) -> p h w", w=D + 1)
                hw = C // 2  # 384 = 6 heads
                ps = ps_proj.tile([128, hw], F32, tag="proj", name="ps_v")
                for ki in range(NC_CHUNKS):
                    nc.tensor.matmul(
                        ps[:],
                        xt[ki][:, sc * 128 : (sc + 1) * 128],
                        wv_sb[ki][:, half * hw : (half + 1) * hw],
                        start=(ki == 0),
                        stop=(ki == NC_CHUNKS - 1),
                    )
                nc.vector.tensor_add(
                    vv[:, half * 6 : (half + 1) * 6, 0:D],
                    ps.rearrange("p (h w) -> p h w", w=D),
                    bvbc.rearrange("p (h w) -> p h w", w=D)[
                        :, half * 6 : (half + 1) * 6, :
                    ],
                )

            def qkv_thunks(xt, q_out, k_out, v_out):
                th = []
                for co in range(NC_CHUNKS):
                    th.append(lambda co=co: qk_group(wq_sb, xt, 0, "q", co, q_out))
                for co in range(NC_CHUNKS):
                    th.append(lambda co=co: qk_group(wk_sb, xt, 6, "k", co, k_out))
                for sc in range(NK_CHUNKS):
                    for half in range(2):
                        th.append(
                            lambda sc=sc, half=half: v_group(xt, sc, half, v_out)
                        )
                return th

            def oproj_thunks(b, cat_sb):
                def one(co):
                    ps = ps_proj.tile([128, S], F32, tag="proj", name="ps_o")
                    for ki in range(NC_CHUNKS):
                        nc.tensor.matmul(
                            ps[:],
                            wo_sb[ki][:, co * 128 : (co + 1) * 128],
                            cat_sb[ki][:],
                            start=(ki == 0),
                            stop=(ki == NC_CHUNKS - 1),
                        )
                    ot = op.tile([128, S], F16, tag=f"o{co}", name="ot")
                    nc.scalar.activation(
                        out=ot[:],
                        in_=ps[:],
                        func=mybir.ActivationFunctionType.Identity,
                        bias=bcol[:, 12 + co : 13 + co],
                    )
                    nc.sync.dma_start(
                        out=y_d.ap()[b, co * 128 : (co + 1) * 128, :], in_=ot[:]
                    )

                return [lambda co=co: one(co) for co in range(NC_CHUNKS)]

            # ---- attention for one batch, weaving `work` thunks between
            # the scores/attn matmul groups -------------------------------
            def attention(b, q_sb, k_sb, v_sb, em_box, work, norm_lag=2):
                cat_sb = []
                for j in range(NC_CHUNKS):
                    ct = cat.tile([128, S], F16, tag=f"c{j}", name=f"cat{j}")
                    cat_sb.append(ct)
                dens = {}
                stgs = {}

                nwork = len(work)
                wi = 0

                def drain(target):
                    nonlocal wi
                    t = min(nwork, target)
                    while wi < t:
                        work[wi]()
                        wi += 1

                def scores_half(hp, khalf, es_pair):
                    # two kc chunks for both heads of the pair; j0 and j1
                    # land in different PSUM buffers (different banks) and
                    # different PE row groups -> the MM pairs overlap.
                    pj0 = ps_s.tile([128, 1024], F32, tag="s", name="ps_s0")
                    pj1 = ps_s.tile([128, 1024], F32, tag="s", name="ps_s1")
                    for i in range(2):
                        kc = 2 * khalf + i
                        for j, ps in ((0, pj0), (1, pj1)):
                            po = j * D
                            nc.tensor.matmul(
                                ps[:, i * 512 : (i + 1) * 512],
                                k_sb[hp][po : po + D, kc * 128 : (kc + 1) * 128],
                                q_sb[hp][po : po + D, :],
                                start=True,
                                stop=True,
                            )
                    for j, ps in ((0, pj0), (1, pj1)):
                        nc.scalar.activation(
                            out=es_pair[j][:, khalf * 1024 : (khalf + 1) * 1024],
                            in_=ps[:],
                            func=mybir.ActivationFunctionType.Exp,
                        )

                def attn_head(h, es):
                    psa = ps_a.tile([D + 1, S], F32, tag="attn", name="psa")
                    for kc in range(NK_CHUNKS):
                        nc.tensor.matmul(
                            psa[:],
                            v_sb[kc][:, h * (D + 1) : (h + 1) * (D + 1)],
                            es[:, kc * 512 : (kc + 1) * 512],
                            start=(kc == 0),
                            stop=(kc == NK_CHUNKS - 1),
                        )
                    # one-op eviction: rows 0-63 unnormalized attn + row 64
                    # denominator; a tiny casting DMA (gpsimd SWDGE is the
                    # only queue whose DMAs may cast) drops the fp16
                    # denominator into a partition-0 fp32 tile
                    stg = wexp.tile([D + 1, S], F16, tag=f"stg{h}", name="stg")
                    # eviction on DVE: each attn_head is emitted BEFORE the
                    # scores_half of its iteration, so the stg lands at the
                    # head of the DVE queue (ahead of the wide muls) and the
                    # single psa bank turns over fast
                    nc.vector.tensor_copy(stg[:], psa[:])
                    # fp16 per-pair denominator tile at base partition 0 (so
                    # the custom-DVE reciprocal is partition-aligned)
                    p, j = h // 2, h % 2
                    if j == 0:
                        dens[p] = rp.tile([2, S], F16, tag=f"den{p % 2}", name="den")
                    nc.sync.dma_start(
                        out=dens[p][j : j + 1, :], in_=stg[D : D + 1, :]
                    )
                    stgs[h] = stg

                def norm_pair(p):
                    # normalize the two heads of pair p into cat chunk p;
                    # woven into the SAME batch so the chain never head-of-
                    # line-blocks the next batch's DVE queue.  One per-pair
                    # reciprocal; rstage DMAs hop each row to partition 0
                    # for the gpsimd broadcast.  Both broadcasts issue
                    # before the first mul so the DVE only eats one
                    # broadcast latency.
                    denf = rp.tile([2, S], F32, tag=f"denf{p % 2}", name="denf")
                    nc.vector.tensor_copy(denf[:], dens[p][:])
                    rip = rp.tile([2, S], F32, tag=f"ri{p % 2}", name="rip")
                    nc.vector.reciprocal_approx_fast(out=rip[:], in_=denf[:])
                    riph = rp.tile([2, S], F16, tag=f"rih{p % 2}", name="riph")
                    nc.vector.tensor_copy(riph[:], rip[:])
                    rbs = []
                    for j in range(2):
                        rstage = rp.tile([1, S], F16, tag="rstage", name="rstage")
                        nc.sync.dma_start(out=rstage[:], in_=riph[j : j + 1, :])
                        rb = rp.tile([64, S], F16, tag="rb", name="rb")
                        nc.gpsimd.partition_broadcast(rb[:], rstage[:])
                        rbs.append(rb)
                    for j in range(2):
                        h = 2 * p + j
                        nc.vector.tensor_mul(
                            cat_sb[p][j * D : j * D + D, :],
                            stgs[h][0:D, :],
                            rbs[j][:],
                        )

                pend = None  # (hp, es0, es1) with muls already emitted
                for hp in range(H // 2):
                    es0 = wexp.tile([128, 4 * S], F16, tag="es0", name="es0")
                    es1 = wexp.tile([128, 4 * S], F16, tag="es1", name="es1")
                    if pend is not None:
                        attn_head(2 * pend[0], pend[1])
                    scores_half(hp, 0, (es0, es1))
                    drain((6 * hp + 2) * nwork // 36)
                    if pend is not None:
                        attn_head(2 * pend[0] + 1, pend[2])
                    scores_half(hp, 1, (es0, es1))
                    nc.vector.tensor_mul(es0[:], es0[:], em_box[0][:])
                    nc.vector.tensor_mul(es1[:], es1[:], em_box[0][:])
                    if hp >= norm_lag:
                        norm_pair(hp - norm_lag)
                    drain((6 * hp + 6) * nwork // 36)
                    pend = (hp, es0, es1)
                attn_head(2 * pend[0], pend[1])
                attn_head(2 * pend[0] + 1, pend[2])
                for p in range(H // 2 - norm_lag, H // 2):
                    norm_pair(p)
                drain(nwork)
                return cat_sb

            # ---- prologue: only what batch 0's first pairs need runs
            # un-interleaved (em, q0/k0/q1/k1, all of V); the rest of batch
            # 0's projections weave into attention(0) itself ---------------
            em_cur = [None]
            q_cur, k_cur, v_cur = {}, {}, {}
            em_thunk(0, em_cur)()
            qkv0 = qkv_thunks(xt0, q_cur, k_cur, v_cur)
            for i in (0, 6, 1, 7):  # q0, k0, q1, k1
                qkv0[i]()
            for th in qkv0[12:]:  # V groups
                th()
            wo_sb.extend(load_w(wo_d, "wo"))

            prev_cat = None
            # late-chunk q/k thunks carried into the NEXT batch's weave so
            # the filler-poor last batch stays fed
            deferred = [qkv0[2], qkv0[8], qkv0[3], qkv0[9], qkv0[4], qkv0[10], qkv0[5], qkv0[11]]
            for b in range(NB):
                work = list(deferred)
                deferred = []
                em_next = [None]
                q_next, k_next, v_next = {}, {}, {}
                if b + 1 < NB:
                    xt_next = load_x(b + 1)
                    qkv = qkv_thunks(xt_next, q_next, k_next, v_next)
                    # q0..2/k0..2 (needed earliest by batch b+1's scores),
                    # em in the middle, V groups after; q3..5/k3..5 are
                    # deferred into batch b+1's own weave (they drain there
                    # before pair 3 needs them)
                    for i in range(3):
                        work += [qkv[i], qkv[6 + i]]
                    work.append(em_thunk(b + 1, em_next))
                    work += qkv[12:]
                    deferred = [
                        qkv[3], qkv[9], qkv[4], qkv[10], qkv[5], qkv[11]
                    ]
                if prev_cat is not None:
                    work += oproj_thunks(b - 1, prev_cat)
                prev_cat = attention(
                    b, q_cur, k_cur, v_cur, em_cur, work,
                    norm_lag=2 if b + 1 < NB else 1,
                )
                em_cur, q_cur, k_cur, v_cur = em_next, q_next, k_next, v_next

            for th in oproj_thunks(NB - 1, prev_cat):
                th()

    nc.compile()
    return nc


def _get_compiled():
    global _COMPILED
    if _COMPILED is None:
        _COMPILED = _build()
    return _COMPILED


def _headmajor(wT):
    """Permute the output-channel axis of a transposed weight from the
    reference's head-minor order (c = d*H + h) to head-major (c' = h*D + d)."""
    return np.ascontiguousarray(
        wT.reshape(C, D, H).transpose(0, 2, 1).reshape(C, C)
    )


def _headmajor_b(bv):
    return np.ascontiguousarray(bv.reshape(D, H).T.reshape(C))


def prepare_in_maps(hidden_state, mask, Wq, bq, Wk, bk, Wv, bv, Wo, bo):
    x = np.asarray(hidden_state).reshape(B, C, S)
    m = np.asarray(mask).reshape(B, S, S)
    scale = np.float32(D**-0.5)

    wqt = np.ascontiguousarray(
        (_headmajor(np.asarray(Wq).T).astype(np.float32) * scale).astype(np.float16)
    )
    wkt = _headmajor(np.asarray(Wk).T)
    wvt = _headmajor(np.asarray(Wv).T)
    wot = np.ascontiguousarray(np.asarray(Wo).T)

    bq_s = (_headmajor_b(np.asarray(bq)).astype(np.float32) * scale).astype(np.float16)
    bk_p = _headmajor_b(np.asarray(bk))
    bo_p = np.asarray(bo)
    bcols = np.stack(
        [bq_s[j * 128 : (j + 1) * 128] for j in range(NC_CHUNKS)]
        + [bk_p[j * 128 : (j + 1) * 128] for j in range(NC_CHUNKS)]
        + [bo_p[j * 128 : (j + 1) * 128] for j in range(NC_CHUNKS)],
        axis=1,
    ).astype(np.float32)
    bvbc = np.ascontiguousarray(
        np.broadcast_to(
            _headmajor_b(np.asarray(bv)).astype(np.float32)[None, :], (128, C)
        )
    )

    shared = {
        "wqt": wqt,
        "wkt": wkt,
        "wvt": wvt,
        "wot": wot,
        "bcols": np.ascontiguousarray(bcols),
        "bvbc": bvbc,
    }
    in_maps = []
    for i in range(NCORES):
        sl = slice(i * NB, (i + 1) * NB)
        in_maps.append(
            dict(
                shared,
                x=np.ascontiguousarray(x[sl]),
                mask=np.ascontiguousarray(m[sl]),
            )
        )
    return in_maps


def kernel(**inputs):
    nc = _get_compiled()
    in_maps = prepare_in_maps(**inputs)
    res = run_bass_kernel_spmd(nc, in_maps, core_ids=list(range(NCORES)))
    y = np.concatenate([res.results[i]["y"] for i in range(NCORES)], axis=0)
    return y.reshape(B, C, 1, S)
